# revision 15
# baseline (speedup 1.0000x reference)
"""Multi-head self-attention TRN2 kernel.

Sharding: tensor-parallel over the 8 heads, one head per NeuronCore.
Each core computes, for its head h:
  Q = X @ (Wq[:, h]/8) + bq/8     (feature-major "qaug" [65, T]; row 64 = ones)
  K = X @ Wk[:, h] + bk           (feature-major "kaug" [65, T]; row 64 = mask*-1e9)
  logits[q,k] = qaug.T @ kaug     (the augmented row adds the mask bias)
  attn = exp(logits) / rowsum     (no max-subtraction needed: |logits| ~ N(0,1))
  ctxT[d,q] = sum_k V[k,d] exp(logitsT[k,q])   (transposed logits recomputed on PE)
  po = (ctxT.T @ Wo[h]) * inv_rowsum           (unnormalized ctx, normalized at the end)
Host: stacks per-core attn slices into [B,H,S,S], sums per-core po partials (+bo).

Matmul operands are float32r (full-rate PE mode for 4-byte floats; PSUM
accumulation fp32; measured end-to-end rel err ~6e-4). float32r-consumed
tiles must be produced by compute instructions that round to f32r, hence the
fp32 staging + copy for DMA-loaded weights/mask.

Phase order A (QKV) -> C (context) -> B (attn output) -> D (projection) keeps
the tensor engine on dense matmul streams early (HAM stays at full clock) and
leaves the DMA-heavy attn writeback for last, overlapped with the projection.
"""

import math
from contextlib import ExitStack

import numpy as np

import concourse.bass as bass
import concourse.mybir as mybir
import concourse.tile as tile
from concourse import bacc
from concourse import bass_utils
from concourse.masks import make_identity

B, S, DIM, H = 2, 2048, 512, 8
DEPTH = DIM // H  # 64
T = B * S  # 4096
NT = T // 128  # 32 token tiles
F32 = mybir.dt.float32
F32R = mybir.dt.float32r
AF = mybir.ActivationFunctionType
MMDT = F32R


def _emit(tc, io):
    nc = tc.nc
    X, mneg = io["x"], io["mneg"]
    attn, po = io["attn"], io["po"]

    with ExitStack() as ctx:
        cp = ctx.enter_context(tc.tile_pool(name="cp", bufs=1))
        qaug = cp.tile([DEPTH + 1, T], MMDT, name="qaug")
        kaug = cp.tile([DEPTH + 1, T], MMDT, name="kaug")
        vsb = cp.tile([128, NT * DEPTH], MMDT, name="vsb")  # token-major V
        ctxT = cp.tile([DEPTH, T], MMDT, name="ctxT")
        inva = cp.tile([128, NT], F32, name="inva")  # 1/rowsum per t-tile
        wosb = cp.tile([DEPTH, DIM], MMDT, name="wosb")
        ident = cp.tile([128, 128], F32, name="ident")

        make_identity(nc, ident)

        # ---- Phase A: X^T via PE transpose; Q^T/8, K^T feature-major; V token-major
        with ExitStack() as actx:
            awp = actx.enter_context(tc.tile_pool(name="awp", bufs=1))
            wqst = awp.tile([128, 4 * DEPTH], F32, name="wqst")
            wkst = awp.tile([128, 4 * DEPTH], F32, name="wkst")
            wvst = awp.tile([128, 4 * DEPTH], F32, name="wvst")
            wqsb = awp.tile([128, 4 * DEPTH], MMDT, name="wqsb")
            wksb = awp.tile([128, 4 * DEPTH], MMDT, name="wksb")
            wvsb = awp.tile([128, 4 * DEPTH], MMDT, name="wvsb")
            bqsb = awp.tile([DEPTH, 1], F32, name="bqsb")
            bksb = awp.tile([DEPTH, 1], F32, name="bksb")
            bvsb = awp.tile([DEPTH, 1], F32, name="bvsb")
            mnegst = awp.tile([1, T], F32, name="mnegst")
            onest = awp.tile([1, T], F32, name="onest")
            wost = awp.tile([DEPTH, DIM], F32, name="wost")
            nc.gpsimd.memset(onest[:], 1.0)
            nc.scalar.copy(qaug[DEPTH : DEPTH + 1, :], onest[:])
            for b in range(B):
                nc.sync.dma_start(
                    mnegst[0:1, b * S : (b + 1) * S], mneg[b : b + 1, :]
                )
            nc.scalar.copy(kaug[DEPTH : DEPTH + 1, :], mnegst[:])
            nc.sync.dma_start(wost[:], io["wo"][:])
            nc.scalar.copy(wosb[:], wost[:])
            xtp = actx.enter_context(tc.tile_pool(name="xtp", bufs=1))
            vtp = actx.enter_context(tc.tile_pool(name="vtp", bufs=1))
            xin = actx.enter_context(tc.tile_pool(name="xin", bufs=4))
            pst = actx.enter_context(tc.tile_pool(name="pst", bufs=2, space="PSUM"))
            psq = actx.enter_context(tc.tile_pool(name="psq", bufs=2, space="PSUM"))

            for j in range(4):
                cs = slice(j * 128, (j + 1) * 128)
                dsl = slice(j * DEPTH, (j + 1) * DEPTH)
                nc.sync.dma_start(wqst[:, dsl], io["wq"][cs, :])
                nc.sync.dma_start(wkst[:, dsl], io["wk"][cs, :])
                nc.sync.dma_start(wvst[:, dsl], io["wv"][cs, :])
            nc.sync.dma_start(bqsb[:], io["bq"][:])
            nc.sync.dma_start(bksb[:], io["bk"][:])
            nc.sync.dma_start(bvsb[:], io["bv"][:])
            nc.scalar.copy(wqsb[:], wqst[:])
            nc.scalar.copy(wksb[:], wkst[:])
            nc.scalar.copy(wvsb[:], wvst[:])

            xt = xtp.tile([128, 4 * T], MMDT, name="xt")  # X^T: c-tile j at cols j*T
            xt_r = xt.rearrange("p (j t) -> p j t", j=4)
            vt = vtp.tile([DEPTH, T], F32, name="vt")  # V^T feature-major

            for i in range(NT):
                xi = xin.tile([128, DIM], F32, name="xi")
                nc.sync.dma_start(xi[:], X[i * 128 : (i + 1) * 128, :])
                pt = pst.tile([128, 512], F32, name="pt")
                for j in range(4):
                    nc.tensor.transpose(
                        pt[:, j * 128 : (j + 1) * 128],
                        xi[:, j * 128 : (j + 1) * 128],
                        ident[:],
                    )
                nc.vector.tensor_copy(
                    xt_r[:, :, i * 128 : (i + 1) * 128],
                    pt.rearrange("p (j t) -> p j t", j=4),
                )

            for wsb, bsb, dest in (
                (wqsb, bqsb, qaug),
                (wksb, bksb, kaug),
                (wvsb, bvsb, vt),
            ):
                for ch in range(T // 512):
                    pq = psq.tile([DEPTH, 512], F32, name="pq")
                    for j in range(4):
                        nc.tensor.matmul(
                            pq[:],
                            wsb[:, j * DEPTH : (j + 1) * DEPTH],
                            xt_r[:, j, ch * 512 : (ch + 1) * 512],
                            start=(j == 0),
                            stop=(j == 3),
                        )
                    nc.scalar.activation(
                        dest[0:DEPTH, ch * 512 : (ch + 1) * 512],
                        pq[:],
                        AF.Identity,
                        bias=bsb[:, 0:1],
                    )

            # V^T -> token-major V tiles
            for i in range(NT):
                pv = pst.tile([128, DEPTH], F32, name="pv")
                nc.tensor.transpose(
                    pv[:], vt[:, i * 128 : (i + 1) * 128], ident[0:DEPTH, 0:DEPTH]
                )
                nc.vector.tensor_copy(vsb[:, i * DEPTH : (i + 1) * DEPTH], pv[:])

        # ---- Phases B+C merged: interleave the DMA/ACT-heavy attn writeback
        # rows with the PE-dense context accumulation so no engine idles.
        with ExitStack() as bctx:
            psb = bctx.enter_context(tc.tile_pool(name="psb", bufs=1, space="PSUM"))
            psa = bctx.enter_context(tc.tile_pool(name="psa", bufs=1, space="PSUM"))
            psc = bctx.enter_context(tc.tile_pool(name="psc", bufs=2, space="PSUM"))
            atp = bctx.enter_context(tc.tile_pool(name="atp", bufs=4))
            acp = bctx.enter_context(tc.tile_pool(name="acp", bufs=6))
            etp = bctx.enter_context(tc.tile_pool(name="etp", bufs=3))
            pc = None
            for b in range(B):
                for qt in range(S // 128):
                    i = b * (S // 128) + qt
                    # B: one attn row-tile [128, 2048]
                    pl = psb.tile([128, S], F32, name="pl")
                    for kc in range(S // 512):
                        nc.tensor.matmul(
                            pl[:, kc * 512 : (kc + 1) * 512],
                            qaug[:, i * 128 : (i + 1) * 128],
                            kaug[:, b * S + kc * 512 : b * S + (kc + 1) * 512],
                            start=True,
                            stop=True,
                        )
                    at = atp.tile([128, S], F32, name="at")
                    sm = acp.tile([128, 1], F32, name="sm")
                    nc.scalar.activation(at[:], pl[:], AF.Exp, accum_out=sm[:])
                    nc.vector.reciprocal(inva[:, i : i + 1], sm[:])
                    nc.vector.tensor_scalar_mul(at[:], at[:], inva[:, i : i + 1])
                    nc.sync.dma_start(
                        attn[b : b + 1, qt * 128 : (qt + 1) * 128, :], at[:]
                    )
                    # C: two transposed-logit pair units
                    for u2 in range(2):
                        j = 2 * qt + u2
                        qc, kt2 = divmod(j, 8)
                        q0 = b * S + qc * 512
                        if kt2 == 0:
                            pc = psc.tile([DEPTH, 512], F32, name="pc")
                        pa = psa.tile([128, 1024], F32, name="pa")
                        for u in range(2):
                            kt = 2 * kt2 + u
                            nc.tensor.matmul(
                                pa[:, u * 512 : (u + 1) * 512],
                                kaug[:, b * S + kt * 128 : b * S + (kt + 1) * 128],
                                qaug[:, q0 : q0 + 512],
                                start=True,
                                stop=True,
                            )
                        et = etp.tile([128, 1024], MMDT, name="et")
                        nc.scalar.activation(et[:], pa[:], AF.Exp)
                        for u in range(2):
                            kt = 2 * kt2 + u
                            g = b * (S // 128) + kt
                            nc.tensor.matmul(
                                pc[:],
                                vsb[:, g * DEPTH : (g + 1) * DEPTH],
                                et[:, u * 512 : (u + 1) * 512],
                                start=(kt == 0),
                                stop=(kt == S // 128 - 1),
                            )
                        if kt2 == 7:
                            nc.vector.tensor_copy(ctxT[:, q0 : q0 + 512], pc[:])

        # ---- Phase D: po = (ctxT.T @ Wo) * inv_rowsum
        with ExitStack() as dctx:
            psd = dctx.enter_context(tc.tile_pool(name="psd", bufs=2, space="PSUM"))
            pop = dctx.enter_context(tc.tile_pool(name="pop", bufs=3))
            for i in range(NT):
                pd = psd.tile([128, DIM], F32, name="pd")
                nc.tensor.matmul(
                    pd[:],
                    ctxT[:, i * 128 : (i + 1) * 128],
                    wosb[:],
                    start=True,
                    stop=True,
                )
                pot = pop.tile([128, DIM], F32, name="pot")
                nc.vector.tensor_scalar_mul(pot[:], pd[:], inva[:, i : i + 1])
                nc.sync.dma_start(po[i * 128 : (i + 1) * 128, :], pot[:])


def _build():
    nc = bacc.Bacc("TRN2", debug=False, enable_asserts=False, num_devices=H)
    io = {}
    io["x"] = nc.dram_tensor("x", [T, DIM], F32, kind="ExternalInput").ap()
    io["mneg"] = nc.dram_tensor("mneg", [B, S], F32, kind="ExternalInput").ap()
    for n in ("wq", "wk", "wv"):
        io[n] = nc.dram_tensor(n, [DIM, DEPTH], F32, kind="ExternalInput").ap()
    io["wo"] = nc.dram_tensor("wo", [DEPTH, DIM], F32, kind="ExternalInput").ap()
    for n in ("bq", "bk", "bv"):
        io[n] = nc.dram_tensor(n, [DEPTH, 1], F32, kind="ExternalInput").ap()
    io["attn"] = nc.dram_tensor("attn", [B, S, S], F32, kind="ExternalOutput").ap()
    io["po"] = nc.dram_tensor("po", [T, DIM], F32, kind="ExternalOutput").ap()
    with tile.TileContext(nc) as tc:
        _emit(tc, io)
    nc.compile()
    return nc


_CACHE = {}
LAST_RESULT = None


def kernel(X, mask, Wq, bq, Wk, bk, Wv, bv, Wo, bo):
    global LAST_RESULT
    scale = np.float32(1.0 / math.sqrt(DEPTH))
    Xf = np.ascontiguousarray(np.asarray(X, np.float32).reshape(T, DIM))
    mneg = np.ascontiguousarray(np.asarray(mask, np.float32) * np.float32(-1e9))
    Wq, Wk, Wv, Wo = (np.asarray(w, np.float32) for w in (Wq, Wk, Wv, Wo))
    bq, bk, bv, bo = (np.asarray(b_, np.float32) for b_ in (bq, bk, bv, bo))

    in_maps = []
    for h in range(H):
        sl = slice(h * DEPTH, (h + 1) * DEPTH)
        in_maps.append(
            {
                "x": Xf,
                "mneg": mneg,
                "wq": np.ascontiguousarray(Wq[:, sl] * scale),
                "wk": np.ascontiguousarray(Wk[:, sl]),
                "wv": np.ascontiguousarray(Wv[:, sl]),
                "wo": np.ascontiguousarray(Wo[sl, :]),
                "bq": np.ascontiguousarray((bq[sl] * scale).reshape(DEPTH, 1)),
                "bk": np.ascontiguousarray(bk[sl].reshape(DEPTH, 1)),
                "bv": np.ascontiguousarray(bv[sl].reshape(DEPTH, 1)),
            }
        )

    if "nc" not in _CACHE:
        _CACHE["nc"] = _build()
    res = bass_utils.run_bass_kernel_spmd(_CACHE["nc"], in_maps, core_ids=list(range(H)))
    LAST_RESULT = res

    attn_full = np.stack([r["attn"] for r in res.results], axis=1)  # [B,H,S,S]
    out = sum(r["po"] for r in res.results).reshape(B, S, DIM) + bo
    return out, attn_full


# revision 16
# speedup vs baseline: 1.4313x; 1.4313x over previous
"""Multi-head self-attention TRN2 kernel.

Sharding: tensor-parallel over the 8 heads, one head per NeuronCore.
Each core computes, for its head h:
  Q = X @ (Wq[:, h]/8) + bq/8     (feature-major "qaug" [65, T]; row 64 = ones)
  K = X @ Wk[:, h] + bk           (feature-major "kaug" [65, T]; row 64 = mask*-1e9)
  logits[q,k] = qaug.T @ kaug     (the augmented row adds the mask bias)
  attn = exp(logits) / rowsum     (no max-subtraction needed: |logits| ~ N(0,1))
  ctxT[d,q] = sum_k V[k,d] exp(logitsT[k,q])   (transposed logits recomputed on PE)
  po = (ctxT.T @ Wo[h]) * inv_rowsum           (unnormalized ctx, normalized at the end)
Host: stacks per-core attn slices into [B,H,S,S], sums per-core po partials (+bo).

Matmul operands are float32r (full-rate PE mode for 4-byte floats; PSUM
accumulation fp32; measured end-to-end rel err ~6e-4). float32r-consumed
tiles must be produced by compute instructions that round to f32r, hence the
fp32 staging + copy for DMA-loaded weights/mask.

Phase order A (QKV) -> C (context) -> B (attn output) -> D (projection) keeps
the tensor engine on dense matmul streams early (HAM stays at full clock) and
leaves the DMA-heavy attn writeback for last, overlapped with the projection.
"""

import math
from contextlib import ExitStack

import numpy as np

import concourse.bass as bass
import concourse.mybir as mybir
import concourse.tile as tile
from concourse import bacc
from concourse import bass_utils
from concourse.masks import make_identity

B, S, DIM, H = 2, 2048, 512, 8
DEPTH = DIM // H  # 64
T = B * S  # 4096
NT = T // 128  # 32 token tiles
F32 = mybir.dt.float32
F32R = mybir.dt.float32r
AF = mybir.ActivationFunctionType
MMDT = F32R


def _emit(tc, io):
    nc = tc.nc
    X, mneg = io["x"], io["mneg"]
    attn, po = io["attn"], io["po"]

    with ExitStack() as ctx:
        cp = ctx.enter_context(tc.tile_pool(name="cp", bufs=1))
        qaug = cp.tile([DEPTH + 1, T], MMDT, name="qaug")
        kaug = cp.tile([DEPTH + 1, T], MMDT, name="kaug")
        vsb = cp.tile([128, NT * DEPTH], MMDT, name="vsb")  # token-major V
        ctxT = cp.tile([DEPTH, T], MMDT, name="ctxT")
        inva = cp.tile([128, NT], F32, name="inva")  # 1/rowsum per t-tile
        wosb = cp.tile([DEPTH, DIM], MMDT, name="wosb")
        ident = cp.tile([128, 128], F32, name="ident")

        make_identity(nc, ident)

        # ---- Phase A: X^T via PE transpose; Q^T/8, K^T feature-major; V token-major
        with ExitStack() as actx:
            awp = actx.enter_context(tc.tile_pool(name="awp", bufs=1))
            wqst = awp.tile([128, 4 * DEPTH], F32, name="wqst")
            wkst = awp.tile([128, 4 * DEPTH], F32, name="wkst")
            wvst = awp.tile([128, 4 * DEPTH], F32, name="wvst")
            wqsb = awp.tile([128, 4 * DEPTH], MMDT, name="wqsb")
            wksb = awp.tile([128, 4 * DEPTH], MMDT, name="wksb")
            wvsb = awp.tile([128, 4 * DEPTH], MMDT, name="wvsb")
            bqsb = awp.tile([DEPTH, 1], F32, name="bqsb")
            bksb = awp.tile([DEPTH, 1], F32, name="bksb")
            bvsb = awp.tile([DEPTH, 1], F32, name="bvsb")
            mnegst = awp.tile([1, T], F32, name="mnegst")
            onest = awp.tile([1, T], F32, name="onest")
            wost = awp.tile([DEPTH, DIM], F32, name="wost")
            nc.gpsimd.memset(onest[:], 1.0)
            nc.scalar.copy(qaug[DEPTH : DEPTH + 1, :], onest[:])
            for b in range(B):
                nc.sync.dma_start(
                    mnegst[0:1, b * S : (b + 1) * S], mneg[b : b + 1, :]
                )
            nc.scalar.copy(kaug[DEPTH : DEPTH + 1, :], mnegst[:])
            nc.sync.dma_start(wost[:], io["wo"][:])
            nc.scalar.copy(wosb[:], wost[:])
            xtp = actx.enter_context(tc.tile_pool(name="xtp", bufs=1))
            vtp = actx.enter_context(tc.tile_pool(name="vtp", bufs=1))
            xin = actx.enter_context(tc.tile_pool(name="xin", bufs=4))
            pst = actx.enter_context(tc.tile_pool(name="pst", bufs=2, space="PSUM"))
            psq = actx.enter_context(tc.tile_pool(name="psq", bufs=2, space="PSUM"))

            for j in range(4):
                cs = slice(j * 128, (j + 1) * 128)
                dsl = slice(j * DEPTH, (j + 1) * DEPTH)
                nc.sync.dma_start(wqst[:, dsl], io["wq"][cs, :])
                nc.sync.dma_start(wkst[:, dsl], io["wk"][cs, :])
                nc.sync.dma_start(wvst[:, dsl], io["wv"][cs, :])
            nc.sync.dma_start(bqsb[:], io["bq"][:])
            nc.sync.dma_start(bksb[:], io["bk"][:])
            nc.sync.dma_start(bvsb[:], io["bv"][:])
            nc.scalar.copy(wqsb[:], wqst[:])
            nc.scalar.copy(wksb[:], wkst[:])
            nc.scalar.copy(wvsb[:], wvst[:])

            xt = xtp.tile([128, 4 * T], MMDT, name="xt")  # X^T: c-tile j at cols j*T
            xt_r = xt.rearrange("p (j t) -> p j t", j=4)
            vt = vtp.tile([DEPTH, T], F32, name="vt")  # V^T feature-major

            for i in range(NT):
                xi = xin.tile([128, DIM], F32, name="xi")
                nc.sync.dma_start(xi[:], X[i * 128 : (i + 1) * 128, :])
                pt = pst.tile([128, 512], F32, name="pt")
                for j in range(4):
                    nc.tensor.transpose(
                        pt[:, j * 128 : (j + 1) * 128],
                        xi[:, j * 128 : (j + 1) * 128],
                        ident[:],
                    )
                nc.vector.tensor_copy(
                    xt_r[:, :, i * 128 : (i + 1) * 128],
                    pt.rearrange("p (j t) -> p j t", j=4),
                )

            for wsb, bsb, dest in (
                (wqsb, bqsb, qaug),
                (wksb, bksb, kaug),
                (wvsb, bvsb, vt),
            ):
                for ch in range(T // 512):
                    pq = psq.tile([DEPTH, 512], F32, name="pq")
                    for j in range(4):
                        nc.tensor.matmul(
                            pq[:],
                            wsb[:, j * DEPTH : (j + 1) * DEPTH],
                            xt_r[:, j, ch * 512 : (ch + 1) * 512],
                            start=(j == 0),
                            stop=(j == 3),
                        )
                    nc.scalar.activation(
                        dest[0:DEPTH, ch * 512 : (ch + 1) * 512],
                        pq[:],
                        AF.Identity,
                        bias=bsb[:, 0:1],
                    )

            # V^T -> token-major V tiles
            for i in range(NT):
                pv = pst.tile([128, DEPTH], F32, name="pv")
                nc.tensor.transpose(
                    pv[:], vt[:, i * 128 : (i + 1) * 128], ident[0:DEPTH, 0:DEPTH]
                )
                nc.vector.tensor_copy(vsb[:, i * DEPTH : (i + 1) * DEPTH], pv[:])

        # ---- Phase C: transposed logits, exp, ctxT = V.T @ expT (unnormalized)
        with ExitStack() as cctx:
            psa = cctx.enter_context(tc.tile_pool(name="psa", bufs=2, space="PSUM"))
            psc = cctx.enter_context(tc.tile_pool(name="psc", bufs=2, space="PSUM"))
            etp = cctx.enter_context(tc.tile_pool(name="etp", bufs=3))
            for b in range(B):
                for qc in range(S // 512):
                    q0 = b * S + qc * 512
                    pc = psc.tile([DEPTH, 512], F32, name="pc")
                    for kt2 in range(S // 256):  # two k-tiles per exp batch
                        pa = psa.tile([128, 1024], F32, name="pa")
                        for u in range(2):
                            kt = 2 * kt2 + u
                            nc.tensor.matmul(
                                pa[:, u * 512 : (u + 1) * 512],
                                kaug[:, b * S + kt * 128 : b * S + (kt + 1) * 128],
                                qaug[:, q0 : q0 + 512],
                                start=True,
                                stop=True,
                            )
                        et = etp.tile([128, 1024], MMDT, name="et")
                        nc.scalar.activation(et[:], pa[:], AF.Exp)
                        for u in range(2):
                            kt = 2 * kt2 + u
                            g = b * (S // 128) + kt
                            nc.tensor.matmul(
                                pc[:],
                                vsb[:, g * DEPTH : (g + 1) * DEPTH],
                                et[:, u * 512 : (u + 1) * 512],
                                start=(kt == 0),
                                stop=(kt == S // 128 - 1),
                            )
                    nc.vector.tensor_copy(ctxT[:, q0 : q0 + 512], pc[:])

        # ---- Phase B: logits [q,k], softmax along free dim, write attn
        with ExitStack() as bctx:
            psb = bctx.enter_context(tc.tile_pool(name="psb", bufs=2, space="PSUM"))
            atp = bctx.enter_context(tc.tile_pool(name="atp", bufs=3))
            acp = bctx.enter_context(tc.tile_pool(name="acp", bufs=4))
            for b in range(B):
                for qt in range(S // 128):
                    i = b * (S // 128) + qt
                    pl = psb.tile([128, S], F32, name="pl")
                    for kc in range(S // 512):
                        nc.tensor.matmul(
                            pl[:, kc * 512 : (kc + 1) * 512],
                            qaug[:, i * 128 : (i + 1) * 128],
                            kaug[:, b * S + kc * 512 : b * S + (kc + 1) * 512],
                            start=True,
                            stop=True,
                        )
                    at = atp.tile([128, S], F32, name="at")
                    sm = acp.tile([128, 1], F32, name="sm")
                    nc.scalar.activation(at[:], pl[:], AF.Exp, accum_out=sm[:])
                    nc.vector.reciprocal(inva[:, i : i + 1], sm[:])
                    nc.vector.tensor_scalar_mul(at[:], at[:], inva[:, i : i + 1])
                    nc.sync.dma_start(
                        attn[b : b + 1, qt * 128 : (qt + 1) * 128, :], at[:]
                    )

        # ---- Phase D: po = (ctxT.T @ Wo) * inv_rowsum
        with ExitStack() as dctx:
            psd = dctx.enter_context(tc.tile_pool(name="psd", bufs=2, space="PSUM"))
            pop = dctx.enter_context(tc.tile_pool(name="pop", bufs=3))
            for i in range(NT):
                pd = psd.tile([128, DIM], F32, name="pd")
                nc.tensor.matmul(
                    pd[:],
                    ctxT[:, i * 128 : (i + 1) * 128],
                    wosb[:],
                    start=True,
                    stop=True,
                )
                pot = pop.tile([128, DIM], F32, name="pot")
                nc.vector.tensor_scalar_mul(pot[:], pd[:], inva[:, i : i + 1])
                nc.sync.dma_start(po[i * 128 : (i + 1) * 128, :], pot[:])


def _build():
    nc = bacc.Bacc("TRN2", debug=False, enable_asserts=False, num_devices=H)
    io = {}
    io["x"] = nc.dram_tensor("x", [T, DIM], F32, kind="ExternalInput").ap()
    io["mneg"] = nc.dram_tensor("mneg", [B, S], F32, kind="ExternalInput").ap()
    for n in ("wq", "wk", "wv"):
        io[n] = nc.dram_tensor(n, [DIM, DEPTH], F32, kind="ExternalInput").ap()
    io["wo"] = nc.dram_tensor("wo", [DEPTH, DIM], F32, kind="ExternalInput").ap()
    for n in ("bq", "bk", "bv"):
        io[n] = nc.dram_tensor(n, [DEPTH, 1], F32, kind="ExternalInput").ap()
    io["attn"] = nc.dram_tensor("attn", [B, S, S], F32, kind="ExternalOutput").ap()
    io["po"] = nc.dram_tensor("po", [T, DIM], F32, kind="ExternalOutput").ap()
    with tile.TileContext(nc) as tc:
        _emit(tc, io)
    nc.compile()
    return nc


_CACHE = {}
LAST_RESULT = None


def kernel(X, mask, Wq, bq, Wk, bk, Wv, bv, Wo, bo):
    global LAST_RESULT
    scale = np.float32(1.0 / math.sqrt(DEPTH))
    Xf = np.ascontiguousarray(np.asarray(X, np.float32).reshape(T, DIM))
    mneg = np.ascontiguousarray(np.asarray(mask, np.float32) * np.float32(-1e9))
    Wq, Wk, Wv, Wo = (np.asarray(w, np.float32) for w in (Wq, Wk, Wv, Wo))
    bq, bk, bv, bo = (np.asarray(b_, np.float32) for b_ in (bq, bk, bv, bo))

    in_maps = []
    for h in range(H):
        sl = slice(h * DEPTH, (h + 1) * DEPTH)
        in_maps.append(
            {
                "x": Xf,
                "mneg": mneg,
                "wq": np.ascontiguousarray(Wq[:, sl] * scale),
                "wk": np.ascontiguousarray(Wk[:, sl]),
                "wv": np.ascontiguousarray(Wv[:, sl]),
                "wo": np.ascontiguousarray(Wo[sl, :]),
                "bq": np.ascontiguousarray((bq[sl] * scale).reshape(DEPTH, 1)),
                "bk": np.ascontiguousarray(bk[sl].reshape(DEPTH, 1)),
                "bv": np.ascontiguousarray(bv[sl].reshape(DEPTH, 1)),
            }
        )

    if "nc" not in _CACHE:
        _CACHE["nc"] = _build()
    res = bass_utils.run_bass_kernel_spmd(_CACHE["nc"], in_maps, core_ids=list(range(H)))
    LAST_RESULT = res

    attn_full = np.stack([r["attn"] for r in res.results], axis=1)  # [B,H,S,S]
    out = sum(r["po"] for r in res.results).reshape(B, S, DIM) + bo
    return out, attn_full


# revision 17
# speedup vs baseline: 1.7350x; 1.2122x over previous
"""Multi-head self-attention TRN2 kernel.

Sharding: tensor-parallel over the 8 heads, one head per NeuronCore.
Each core computes, for its head h:
  Q = X @ (Wq[:, h]/8) + bq/8     (feature-major "qaug" [65, T]; row 64 = ones)
  K = X @ Wk[:, h] + bk           (feature-major "kaug" [65, T]; row 64 = mask*-1e9)
  logits[q,k] = qaug.T @ kaug     (the augmented row adds the mask bias)
  attn = exp(logits) / rowsum     (no max-subtraction needed: |logits| ~ N(0,1))
  ctxT[d,q] = sum_k V[k,d] exp(logitsT[k,q])   (transposed logits recomputed on PE)
  po = (ctxT.T @ Wo[h]) * inv_rowsum           (unnormalized ctx, normalized at the end)
Host: stacks per-core attn slices into [B,H,S,S], sums per-core po partials (+bo).

Matmul operands are float32r (full-rate PE mode for 4-byte floats; PSUM
accumulation fp32; measured end-to-end rel err ~6e-4). float32r-consumed
tiles must be produced by compute instructions that round to f32r, hence the
fp32 staging + copy for DMA-loaded weights/mask.

Phase order A (QKV) -> C (context) -> B (attn output) -> D (projection) keeps
the tensor engine on dense matmul streams early (HAM stays at full clock) and
leaves the DMA-heavy attn writeback for last, overlapped with the projection.
"""

import math
from contextlib import ExitStack

import numpy as np

import concourse.bass as bass
import concourse.mybir as mybir
import concourse.tile as tile
from concourse import bacc
from concourse import bass_utils
from concourse.masks import make_identity

B, S, DIM, H = 2, 2048, 512, 8
DEPTH = DIM // H  # 64
T = B * S  # 4096
NT = T // 128  # 32 token tiles
F32 = mybir.dt.float32
F32R = mybir.dt.float32r
AF = mybir.ActivationFunctionType
MMDT = F32R


def _emit(tc, io):
    nc = tc.nc
    X, mneg = io["x"], io["mneg"]
    attn, po = io["attn"], io["po"]

    with ExitStack() as ctx:
        cp = ctx.enter_context(tc.tile_pool(name="cp", bufs=1))
        qaug = cp.tile([DEPTH + 1, T], MMDT, name="qaug")
        kaug = cp.tile([DEPTH + 1, T], MMDT, name="kaug")
        vsb = cp.tile([128, NT * DEPTH], MMDT, name="vsb")  # token-major V
        ctxT = cp.tile([DEPTH, T], MMDT, name="ctxT")
        inva = cp.tile([128, NT], F32, name="inva")  # 1/rowsum per t-tile
        wosb = cp.tile([DEPTH, DIM], MMDT, name="wosb")
        ident = cp.tile([128, 128], F32, name="ident")

        make_identity(nc, ident)

        # ---- Phase A: X^T via PE transpose; Q^T/8, K^T feature-major; V token-major
        with ExitStack() as actx:
            awp = actx.enter_context(tc.tile_pool(name="awp", bufs=1))
            wqsb = awp.tile([128, 4 * DEPTH], MMDT, name="wqsb")
            wksb = awp.tile([128, 4 * DEPTH], MMDT, name="wksb")
            wvsb = awp.tile([128, 4 * DEPTH], MMDT, name="wvsb")
            bqsb = awp.tile([DEPTH, 1], F32, name="bqsb")
            bksb = awp.tile([DEPTH, 1], F32, name="bksb")
            bvsb = awp.tile([DEPTH, 1], F32, name="bvsb")
            onest = awp.tile([1, 512], F32, name="onest")
            nc.gpsimd.memset(onest[:], 1.0)
            for ch in range(T // 512):
                nc.scalar.copy(
                    qaug[DEPTH : DEPTH + 1, ch * 512 : (ch + 1) * 512], onest[:]
                )
            for b in range(B):
                nc.sync.dma_start(
                    kaug[DEPTH : DEPTH + 1, b * S : (b + 1) * S], mneg[b : b + 1, :]
                )
            nc.sync.dma_start(wosb[:], io["wo"][:])
            xtp = actx.enter_context(tc.tile_pool(name="xtp", bufs=1))
            vtp = actx.enter_context(tc.tile_pool(name="vtp", bufs=1))
            pst = actx.enter_context(tc.tile_pool(name="pst", bufs=2, space="PSUM"))
            psq = actx.enter_context(tc.tile_pool(name="psq", bufs=2, space="PSUM"))

            for j in range(4):
                cs = slice(j * 128, (j + 1) * 128)
                dsl = slice(j * DEPTH, (j + 1) * DEPTH)
                nc.sync.dma_start(wqsb[:, dsl], io["wq"][cs, :])
                nc.sync.dma_start(wksb[:, dsl], io["wk"][cs, :])
                nc.sync.dma_start(wvsb[:, dsl], io["wv"][cs, :])
            nc.sync.dma_start(bqsb[:], io["bq"][:])
            nc.sync.dma_start(bksb[:], io["bk"][:])
            nc.sync.dma_start(bvsb[:], io["bv"][:])

            xt = xtp.tile([128, 4 * T], MMDT, name="xt")  # X^T: c-tile j at cols j*T
            xt_r = xt.rearrange("p (j t) -> p j t", j=4)
            vt = vtp.tile([DEPTH, T], F32, name="vt")  # V^T feature-major

            for j in range(4):
                nc.sync.dma_start(xt_r[:, j, :], X[j * 128 : (j + 1) * 128, :])

            for wsb, bsb, dest in (
                (wqsb, bqsb, qaug),
                (wksb, bksb, kaug),
                (wvsb, bvsb, vt),
            ):
                for ch in range(T // 512):
                    pq = psq.tile([DEPTH, 512], F32, name="pq")
                    for j in range(4):
                        nc.tensor.matmul(
                            pq[:],
                            wsb[:, j * DEPTH : (j + 1) * DEPTH],
                            xt_r[:, j, ch * 512 : (ch + 1) * 512],
                            start=(j == 0),
                            stop=(j == 3),
                        )
                    nc.scalar.activation(
                        dest[0:DEPTH, ch * 512 : (ch + 1) * 512],
                        pq[:],
                        AF.Identity,
                        bias=bsb[:, 0:1],
                    )

            # V^T -> token-major V tiles
            for i in range(NT):
                pv = pst.tile([128, DEPTH], F32, name="pv")
                nc.tensor.transpose(
                    pv[:], vt[:, i * 128 : (i + 1) * 128], ident[0:DEPTH, 0:DEPTH]
                )
                nc.vector.tensor_copy(vsb[:, i * DEPTH : (i + 1) * DEPTH], pv[:])

        # ---- Phase C: transposed logits, exp, ctxT = V.T @ expT (unnormalized)
        with ExitStack() as cctx:
            psa = cctx.enter_context(tc.tile_pool(name="psa", bufs=3, space="PSUM"))
            psc = cctx.enter_context(tc.tile_pool(name="psc", bufs=2, space="PSUM"))
            etp = cctx.enter_context(tc.tile_pool(name="etp", bufs=3))
            for b in range(B):
                for qc in range(S // 512):
                    q0 = b * S + qc * 512
                    pc = psc.tile([DEPTH, 512], F32, name="pc")
                    for kt2 in range(S // 256):  # two k-tiles per exp batch
                        pa = psa.tile([128, 1024], F32, name="pa")
                        for u in range(2):
                            kt = 2 * kt2 + u
                            nc.tensor.matmul(
                                pa[:, u * 512 : (u + 1) * 512],
                                kaug[:, b * S + kt * 128 : b * S + (kt + 1) * 128],
                                qaug[:, q0 : q0 + 512],
                                start=True,
                                stop=True,
                            )
                        et = etp.tile([128, 1024], MMDT, name="et")
                        nc.scalar.activation(et[:], pa[:], AF.Exp)
                        for u in range(2):
                            kt = 2 * kt2 + u
                            g = b * (S // 128) + kt
                            nc.tensor.matmul(
                                pc[:],
                                vsb[:, g * DEPTH : (g + 1) * DEPTH],
                                et[:, u * 512 : (u + 1) * 512],
                                start=(kt == 0),
                                stop=(kt == S // 128 - 1),
                            )
                    nc.vector.tensor_copy(ctxT[:, q0 : q0 + 512], pc[:])

        # ---- Phase B: logits [q,k], softmax along free dim, write attn
        with ExitStack() as bctx:
            psb = bctx.enter_context(tc.tile_pool(name="psb", bufs=2, space="PSUM"))
            atp = bctx.enter_context(tc.tile_pool(name="atp", bufs=4))
            acp = bctx.enter_context(tc.tile_pool(name="acp", bufs=4))
            for b in range(B):
                for qt in range(S // 128):
                    i = b * (S // 128) + qt
                    pl = psb.tile([128, S], F32, name="pl")
                    for kc in range(S // 512):
                        nc.tensor.matmul(
                            pl[:, kc * 512 : (kc + 1) * 512],
                            qaug[:, i * 128 : (i + 1) * 128],
                            kaug[:, b * S + kc * 512 : b * S + (kc + 1) * 512],
                            start=True,
                            stop=True,
                        )
                    at = atp.tile([128, S], F32, name="at")
                    sm = acp.tile([128, 1], F32, name="sm")
                    nc.scalar.activation(at[:], pl[:], AF.Exp, accum_out=sm[:])
                    nc.vector.reciprocal(inva[:, i : i + 1], sm[:])
                    nc.vector.tensor_scalar_mul(at[:], at[:], inva[:, i : i + 1])
                    nc.sync.dma_start(
                        attn[b : b + 1, qt * 128 : (qt + 1) * 128, :], at[:]
                    )

        # ---- Phase D: po = (ctxT.T @ Wo) * inv_rowsum
        with ExitStack() as dctx:
            psd = dctx.enter_context(tc.tile_pool(name="psd", bufs=2, space="PSUM"))
            pop = dctx.enter_context(tc.tile_pool(name="pop", bufs=4))
            for i in range(NT):
                pd = psd.tile([128, DIM], F32, name="pd")
                nc.tensor.matmul(
                    pd[:],
                    ctxT[:, i * 128 : (i + 1) * 128],
                    wosb[:],
                    start=True,
                    stop=True,
                )
                pot = pop.tile([128, DIM], F32, name="pot")
                nc.vector.tensor_scalar_mul(pot[:], pd[:], inva[:, i : i + 1])
                nc.sync.dma_start(po[i * 128 : (i + 1) * 128, :], pot[:])


def _build():
    nc = bacc.Bacc("TRN2", debug=False, enable_asserts=False, num_devices=H)
    io = {}
    io["x"] = nc.dram_tensor("x", [DIM, T], MMDT, kind="ExternalInput").ap()
    io["mneg"] = nc.dram_tensor("mneg", [B, S], MMDT, kind="ExternalInput").ap()
    for n in ("wq", "wk", "wv"):
        io[n] = nc.dram_tensor(n, [DIM, DEPTH], MMDT, kind="ExternalInput").ap()
    io["wo"] = nc.dram_tensor("wo", [DEPTH, DIM], MMDT, kind="ExternalInput").ap()
    for n in ("bq", "bk", "bv"):
        io[n] = nc.dram_tensor(n, [DEPTH, 1], F32, kind="ExternalInput").ap()
    io["attn"] = nc.dram_tensor("attn", [B, S, S], F32, kind="ExternalOutput").ap()
    io["po"] = nc.dram_tensor("po", [T, DIM], F32, kind="ExternalOutput").ap()
    with tile.TileContext(nc) as tc:
        _emit(tc, io)
    nc.compile()
    return nc


_CACHE = {}
LAST_RESULT = None


def kernel(X, mask, Wq, bq, Wk, bk, Wv, bv, Wo, bo):
    global LAST_RESULT
    scale = np.float32(1.0 / math.sqrt(DEPTH))
    Xf = np.ascontiguousarray(np.asarray(X, np.float32).reshape(T, DIM).T)
    mneg = np.ascontiguousarray(np.asarray(mask, np.float32) * np.float32(-1e9))
    Wq, Wk, Wv, Wo = (np.asarray(w, np.float32) for w in (Wq, Wk, Wv, Wo))
    bq, bk, bv, bo = (np.asarray(b_, np.float32) for b_ in (bq, bk, bv, bo))

    in_maps = []
    for h in range(H):
        sl = slice(h * DEPTH, (h + 1) * DEPTH)
        in_maps.append(
            {
                "x": Xf,
                "mneg": mneg,
                "wq": np.ascontiguousarray(Wq[:, sl] * scale),
                "wk": np.ascontiguousarray(Wk[:, sl]),
                "wv": np.ascontiguousarray(Wv[:, sl]),
                "wo": np.ascontiguousarray(Wo[sl, :]),
                "bq": np.ascontiguousarray((bq[sl] * scale).reshape(DEPTH, 1)),
                "bk": np.ascontiguousarray(bk[sl].reshape(DEPTH, 1)),
                "bv": np.ascontiguousarray(bv[sl].reshape(DEPTH, 1)),
            }
        )

    if "nc" not in _CACHE:
        _CACHE["nc"] = _build()
    res = bass_utils.run_bass_kernel_spmd(_CACHE["nc"], in_maps, core_ids=list(range(H)))
    LAST_RESULT = res

    attn_full = np.stack([r["attn"] for r in res.results], axis=1)  # [B,H,S,S]
    out = sum(r["po"] for r in res.results).reshape(B, S, DIM) + bo
    return out, attn_full


# revision 19
# speedup vs baseline: 1.7522x; 1.0099x over previous
"""Multi-head self-attention TRN2 kernel.

Sharding: tensor-parallel over the 8 heads, one head per NeuronCore.
Each core computes, for its head h:
  Q = X @ (Wq[:, h]/8) + bq/8     (feature-major "qaug" [65, T]; row 64 = ones)
  K = X @ Wk[:, h] + bk           (feature-major "kaug" [65, T]; row 64 = mask*-1e9)
  logits[q,k] = qaug.T @ kaug     (the augmented row adds the mask bias)
  attn = exp(logits) / rowsum     (no max-subtraction needed: |logits| ~ N(0,1))
  ctxT[d,q] = sum_k V[k,d] exp(logitsT[k,q])   (transposed logits recomputed on PE)
  po = (ctxT.T @ Wo[h]) * inv_rowsum           (unnormalized ctx, normalized at the end)
Host: stacks per-core attn slices into [B,H,S,S], sums per-core po partials (+bo).

Matmul operands are float32r (full-rate PE mode for 4-byte floats; PSUM
accumulation fp32; measured end-to-end rel err ~6e-4). float32r-consumed
tiles must be produced by compute instructions that round to f32r, hence the
fp32 staging + copy for DMA-loaded weights/mask.

Phase order A (QKV) -> C (context) -> B (attn output) -> D (projection) keeps
the tensor engine on dense matmul streams early (HAM stays at full clock) and
leaves the DMA-heavy attn writeback for last, overlapped with the projection.
"""

import math
from contextlib import ExitStack

import numpy as np

import concourse.bass as bass
import concourse.mybir as mybir
import concourse.tile as tile
from concourse import bacc
from concourse import bass_utils
from concourse.masks import make_identity

B, S, DIM, H = 2, 2048, 512, 8
DEPTH = DIM // H  # 64
T = B * S  # 4096
NT = T // 128  # 32 token tiles
F32 = mybir.dt.float32
F32R = mybir.dt.float32r
AF = mybir.ActivationFunctionType
MMDT = F32R


def _emit(tc, io):
    nc = tc.nc
    X, mneg = io["x"], io["mneg"]
    attn, po = io["attn"], io["po"]

    with ExitStack() as ctx:
        cp = ctx.enter_context(tc.tile_pool(name="cp", bufs=1))
        qaug = cp.tile([DEPTH + 1, T], MMDT, name="qaug")
        kaug = cp.tile([DEPTH + 1, T], MMDT, name="kaug")
        vsb = cp.tile([128, NT * DEPTH], MMDT, name="vsb")  # token-major V
        ctxT = cp.tile([DEPTH, T], MMDT, name="ctxT")
        inva = cp.tile([128, NT], F32, name="inva")  # 1/rowsum per t-tile
        wosb = cp.tile([DEPTH, DIM], MMDT, name="wosb")
        ident = cp.tile([128, 128], F32, name="ident")

        make_identity(nc, ident)

        # ---- Phase A: X^T via PE transpose; Q^T/8, K^T feature-major; V token-major
        with ExitStack() as actx:
            awp = actx.enter_context(tc.tile_pool(name="awp", bufs=1))
            wqsb = awp.tile([128, 4 * DEPTH], MMDT, name="wqsb")
            wksb = awp.tile([128, 4 * DEPTH], MMDT, name="wksb")
            wvsb = awp.tile([128, 4 * DEPTH], MMDT, name="wvsb")
            bqsb = awp.tile([DEPTH, 1], F32, name="bqsb")
            bksb = awp.tile([DEPTH, 1], F32, name="bksb")
            bvsb = awp.tile([DEPTH, 1], F32, name="bvsb")
            onest = awp.tile([1, 512], F32, name="onest")
            nc.gpsimd.memset(onest[:], 1.0)
            for ch in range(T // 512):
                nc.scalar.copy(
                    qaug[DEPTH : DEPTH + 1, ch * 512 : (ch + 1) * 512], onest[:]
                )
            for b in range(B):
                nc.sync.dma_start(
                    kaug[DEPTH : DEPTH + 1, b * S : (b + 1) * S], mneg[b : b + 1, :]
                )
            nc.sync.dma_start(wosb[:], io["wo"][:])
            xtp = actx.enter_context(tc.tile_pool(name="xtp", bufs=1))
            vtp = actx.enter_context(tc.tile_pool(name="vtp", bufs=1))
            pst = actx.enter_context(tc.tile_pool(name="pst", bufs=2, space="PSUM"))
            psq = actx.enter_context(tc.tile_pool(name="psq", bufs=2, space="PSUM"))

            for j in range(4):
                cs = slice(j * 128, (j + 1) * 128)
                dsl = slice(j * DEPTH, (j + 1) * DEPTH)
                nc.sync.dma_start(wqsb[:, dsl], io["wq"][cs, :])
                nc.sync.dma_start(wksb[:, dsl], io["wk"][cs, :])
                nc.sync.dma_start(wvsb[:, dsl], io["wv"][cs, :])
            nc.sync.dma_start(bqsb[:], io["bq"][:])
            nc.sync.dma_start(bksb[:], io["bk"][:])
            nc.sync.dma_start(bvsb[:], io["bv"][:])

            xt = xtp.tile([128, 4 * T], MMDT, name="xt")  # X^T: c-tile j at cols j*T
            xt_r = xt.rearrange("p (j t) -> p j t", j=4)
            vt = vtp.tile([DEPTH, T], F32, name="vt")  # V^T feature-major

            # token-range-major chunked loads: the first QKV matmuls only
            # need the first 512-token range, so the PE starts ~6us in
            # instead of waiting ~23us for four serialized 2MB transfers.
            for tr in range(8):
                tsl = slice(tr * 512, (tr + 1) * 512)
                for j in range(4):
                    nc.sync.dma_start(
                        xt_r[:, j, tsl], X[j * 128 : (j + 1) * 128, tsl]
                    )

            for wsb, bsb, dest in (
                (wqsb, bqsb, qaug),
                (wksb, bksb, kaug),
                (wvsb, bvsb, vt),
            ):
                for ch in range(T // 512):
                    pq = psq.tile([DEPTH, 512], F32, name="pq")
                    for j in range(4):
                        nc.tensor.matmul(
                            pq[:],
                            wsb[:, j * DEPTH : (j + 1) * DEPTH],
                            xt_r[:, j, ch * 512 : (ch + 1) * 512],
                            start=(j == 0),
                            stop=(j == 3),
                        )
                    nc.scalar.activation(
                        dest[0:DEPTH, ch * 512 : (ch + 1) * 512],
                        pq[:],
                        AF.Identity,
                        bias=bsb[:, 0:1],
                    )

            # V^T -> token-major V tiles
            for i in range(NT):
                pv = pst.tile([128, DEPTH], F32, name="pv")
                nc.tensor.transpose(
                    pv[:], vt[:, i * 128 : (i + 1) * 128], ident[0:DEPTH, 0:DEPTH]
                )
                nc.vector.tensor_copy(vsb[:, i * DEPTH : (i + 1) * DEPTH], pv[:])

        # ---- Phase C: transposed logits, exp, ctxT = V.T @ expT (unnormalized)
        with ExitStack() as cctx:
            psa = cctx.enter_context(tc.tile_pool(name="psa", bufs=3, space="PSUM"))
            psc = cctx.enter_context(tc.tile_pool(name="psc", bufs=2, space="PSUM"))
            etp = cctx.enter_context(tc.tile_pool(name="etp", bufs=3))
            for b in range(B):
                for qc in range(S // 512):
                    q0 = b * S + qc * 512
                    pc = psc.tile([DEPTH, 512], F32, name="pc")
                    for kt2 in range(S // 256):  # two k-tiles per exp batch
                        pa = psa.tile([128, 1024], F32, name="pa")
                        for u in range(2):
                            kt = 2 * kt2 + u
                            nc.tensor.matmul(
                                pa[:, u * 512 : (u + 1) * 512],
                                kaug[:, b * S + kt * 128 : b * S + (kt + 1) * 128],
                                qaug[:, q0 : q0 + 512],
                                start=True,
                                stop=True,
                            )
                        et = etp.tile([128, 1024], MMDT, name="et")
                        nc.scalar.activation(et[:], pa[:], AF.Exp)
                        for u in range(2):
                            kt = 2 * kt2 + u
                            g = b * (S // 128) + kt
                            nc.tensor.matmul(
                                pc[:],
                                vsb[:, g * DEPTH : (g + 1) * DEPTH],
                                et[:, u * 512 : (u + 1) * 512],
                                start=(kt == 0),
                                stop=(kt == S // 128 - 1),
                            )
                    nc.vector.tensor_copy(ctxT[:, q0 : q0 + 512], pc[:])

        # ---- Phase B: logits [q,k], softmax along free dim, write attn
        with ExitStack() as bctx:
            psb = bctx.enter_context(tc.tile_pool(name="psb", bufs=2, space="PSUM"))
            atp = bctx.enter_context(tc.tile_pool(name="atp", bufs=4))
            acp = bctx.enter_context(tc.tile_pool(name="acp", bufs=4))
            for b in range(B):
                for qt in range(S // 128):
                    i = b * (S // 128) + qt
                    pl = psb.tile([128, S], F32, name="pl")
                    for kc in range(S // 512):
                        nc.tensor.matmul(
                            pl[:, kc * 512 : (kc + 1) * 512],
                            qaug[:, i * 128 : (i + 1) * 128],
                            kaug[:, b * S + kc * 512 : b * S + (kc + 1) * 512],
                            start=True,
                            stop=True,
                        )
                    at = atp.tile([128, S], F32, name="at")
                    sm = acp.tile([128, 1], F32, name="sm")
                    nc.scalar.activation(at[:], pl[:], AF.Exp, accum_out=sm[:])
                    nc.vector.reciprocal(inva[:, i : i + 1], sm[:])
                    nc.vector.tensor_scalar_mul(at[:], at[:], inva[:, i : i + 1])
                    nc.sync.dma_start(
                        attn[b : b + 1, qt * 128 : (qt + 1) * 128, :], at[:]
                    )

        # ---- Phase D: po = (ctxT.T @ Wo) * inv_rowsum
        with ExitStack() as dctx:
            psd = dctx.enter_context(tc.tile_pool(name="psd", bufs=2, space="PSUM"))
            pop = dctx.enter_context(tc.tile_pool(name="pop", bufs=4))
            for i in range(NT):
                pd = psd.tile([128, DIM], F32, name="pd")
                nc.tensor.matmul(
                    pd[:],
                    ctxT[:, i * 128 : (i + 1) * 128],
                    wosb[:],
                    start=True,
                    stop=True,
                )
                pot = pop.tile([128, DIM], F32, name="pot")
                nc.vector.tensor_scalar_mul(pot[:], pd[:], inva[:, i : i + 1])
                nc.sync.dma_start(po[i * 128 : (i + 1) * 128, :], pot[:])


def _build():
    nc = bacc.Bacc("TRN2", debug=False, enable_asserts=False, num_devices=H)
    io = {}
    io["x"] = nc.dram_tensor("x", [DIM, T], MMDT, kind="ExternalInput").ap()
    io["mneg"] = nc.dram_tensor("mneg", [B, S], MMDT, kind="ExternalInput").ap()
    for n in ("wq", "wk", "wv"):
        io[n] = nc.dram_tensor(n, [DIM, DEPTH], MMDT, kind="ExternalInput").ap()
    io["wo"] = nc.dram_tensor("wo", [DEPTH, DIM], MMDT, kind="ExternalInput").ap()
    for n in ("bq", "bk", "bv"):
        io[n] = nc.dram_tensor(n, [DEPTH, 1], F32, kind="ExternalInput").ap()
    io["attn"] = nc.dram_tensor("attn", [B, S, S], F32, kind="ExternalOutput").ap()
    io["po"] = nc.dram_tensor("po", [T, DIM], F32, kind="ExternalOutput").ap()
    with tile.TileContext(nc) as tc:
        _emit(tc, io)
    nc.compile()
    return nc


_CACHE = {}
LAST_RESULT = None


def kernel(X, mask, Wq, bq, Wk, bk, Wv, bv, Wo, bo):
    global LAST_RESULT
    scale = np.float32(1.0 / math.sqrt(DEPTH))
    Xf = np.ascontiguousarray(np.asarray(X, np.float32).reshape(T, DIM).T)
    mneg = np.ascontiguousarray(np.asarray(mask, np.float32) * np.float32(-1e9))
    Wq, Wk, Wv, Wo = (np.asarray(w, np.float32) for w in (Wq, Wk, Wv, Wo))
    bq, bk, bv, bo = (np.asarray(b_, np.float32) for b_ in (bq, bk, bv, bo))

    in_maps = []
    for h in range(H):
        sl = slice(h * DEPTH, (h + 1) * DEPTH)
        in_maps.append(
            {
                "x": Xf,
                "mneg": mneg,
                "wq": np.ascontiguousarray(Wq[:, sl] * scale),
                "wk": np.ascontiguousarray(Wk[:, sl]),
                "wv": np.ascontiguousarray(Wv[:, sl]),
                "wo": np.ascontiguousarray(Wo[sl, :]),
                "bq": np.ascontiguousarray((bq[sl] * scale).reshape(DEPTH, 1)),
                "bk": np.ascontiguousarray(bk[sl].reshape(DEPTH, 1)),
                "bv": np.ascontiguousarray(bv[sl].reshape(DEPTH, 1)),
            }
        )

    if "nc" not in _CACHE:
        _CACHE["nc"] = _build()
    res = bass_utils.run_bass_kernel_spmd(_CACHE["nc"], in_maps, core_ids=list(range(H)))
    LAST_RESULT = res

    attn_full = np.stack([r["attn"] for r in res.results], axis=1)  # [B,H,S,S]
    out = sum(r["po"] for r in res.results).reshape(B, S, DIM) + bo
    return out, attn_full
